# revision 5
# baseline (speedup 1.0000x reference)
"""Multi-head attention (B=4, L=2048, E=1024, H=16, DK=64) on 8 TRN2 cores.

Sharding: core c -> (batch b = c//2, head-group g = c%2 of 8 heads).

Single fused instruction stream per core, organized as one software
pipeline over 512 attention steps (4 query-quarters x 8 heads x 16
key-tiles). Per step: ST matmul (PE) -> exp (ACT) -> AV matmul (PE,
lagging one step). The scores PSUM tile is triple-buffered so ACT never
waits on a WAR hazard; QKV-projection and FC matmul chains are dripped
into the stream as background PE work between attention steps. Each
query-quarter's partial FC output is ReduceScattered (pairwise, bf16)
directly into the output tensor while later quarters compute, so only
the last quarter's FC+RS is exposed. Host casts bf16 -> f32.

Self-contained: hardcodes all shapes; requires only the concourse stack.
"""

import numpy as np
import ml_dtypes

try:
    import axon_prof

    axon_prof.install()
except Exception:
    pass

import concourse.mybir as mybir
import concourse.tile as tile
from concourse import bacc
from concourse import bass_utils

B, L, E = 4, 2048, 1024
H, DK = 16, 64
H8 = 8                      # heads per core
F = H8 * 3 * DK             # qkv features per core = 1536
FO = H8 * DK                # attn-out features per core = 512
NCORES = 8
Q4 = L // 4                 # 512 queries per quarter
Q8 = Q4 // 2                # 256 tokens scattered to each pair member

f32 = mybir.dt.float32
bf16 = mybir.dt.bfloat16
Exp = mybir.ActivationFunctionType.Exp
MUL = mybir.AluOpType.mult
ADD = mybir.AluOpType.add

_CACHE = {}


def build_nc():
    nc = bacc.Bacc("TRN2", target_bir_lowering=False, debug=False, num_devices=NCORES)

    # x arrives already transposed (host-side) so no xbar transpose is needed
    x = nc.dram_tensor("x", [E, L], bf16, kind="ExternalInput")
    w_qkv = nc.dram_tensor("w_qkv", [E, F], bf16, kind="ExternalInput")
    b_qkv = nc.dram_tensor("b_qkv", [128, 12], f32, kind="ExternalInput")
    w_fc = nc.dram_tensor("w_fc", [FO, E], bf16, kind="ExternalInput")
    b_fc = nc.dram_tensor("b_fc", [1, E], f32, kind="ExternalInput")
    # 4 quarters x 256 scattered tokens each; host casts bf16 -> f32
    out = nc.dram_tensor("out", [L // 2, E], bf16, kind="ExternalOutput")

    PAIRS = [[0, 1], [2, 3], [4, 5], [6, 7]]

    with tile.TileContext(nc) as tc:
        with (
            tc.tile_pool(name="persist", bufs=1) as pp,
            tc.tile_pool(name="work", bufs=2) as wp,
            tc.tile_pool(name="ys", bufs=3) as yp_pool,
            tc.tile_pool(name="stp", bufs=3, space="PSUM") as stp,
            tc.tile_pool(name="avp", bufs=2, space="PSUM") as avp,
            tc.tile_pool(name="qp", bufs=3, space="PSUM") as qp,
            tc.tile_pool(name="dram", bufs=1, space="DRAM") as dram,
        ):
            # ---- persistent SBUF ----
            xT = pp.tile([128, 8, L], bf16, tag="xT")          # X^T  4 MiB
            wq = pp.tile([128, 8, F], bf16, tag="wq")          # 3 MiB
            bq = pp.tile([128, 12], f32, tag="bq")
            wfc = pp.tile([128, 4, E], bf16, tag="wfc")        # 1 MiB
            bias = pp.tile([128, E], f32, tag="bias")          # 0.5 MiB
            qt = pp.tile([128, 4, L], bf16, tag="qt")          # Q^T 2 MiB
            kt = pp.tile([128, 4, L], bf16, tag="kt")          # K^T 2 MiB
            vt = pp.tile([128, 4, L], bf16, tag="vt")          # V^T 2 MiB
            # V natural layout, 80-elem stride; col 64 holds the ones column
            # so AV matmuls with lhsT [V|1] (M=65) produce rowsums for free
            v = pp.tile([128, H8, 16, 80], bf16, tag="v")      # 2.5 MiB
            onT = pp.tile([128, 4, L], bf16, tag="onT")        # attn out^T 2 MiB

            rs_in = [
                dram.tile([Q4, E], bf16, name=f"rs_in{i}", tag=f"rs_in{i}")
                for i in range(4)
            ]
            rs_out = [
                dram.tile([Q8, E], bf16, name=f"rs_out{i}", tag=f"rs_out{i}")
                for i in range(4)
            ]

            # ---- input DMAs over both HWDGE queues ----
            queues = [nc.sync, nc.scalar]
            for e in range(8):
                queues[e % 2].dma_start(xT[:, e, :], x[e * 128 : (e + 1) * 128, :])
            for e in range(8):
                queues[(e + 1) % 2].dma_start(
                    wq[:, e, :], w_qkv[e * 128 : (e + 1) * 128, :]
                )
            nc.scalar.dma_start(bq[:], b_qkv[:])
            nc.scalar.dma_start(wfc[:], w_fc.rearrange("(c p) e -> p c e", p=128))
            bfc_row = pp.tile([1, E], f32, tag="bfc_row")
            nc.scalar.dma_start(bfc_row[:], b_fc[:])
            nc.gpsimd.partition_broadcast(bias[:], bfc_row[:])
            nc.vector.memset(v[:, :, :, 64:65], 1.0)

            # ---- instruction emitters ----
            def qkv_chain(ft, tb):
                ps = qp.tile([128, 512], f32, tag="ps", name="ps")
                for kc in range(8):
                    nc.tensor.matmul(
                        ps[:],
                        wq[:, kc, ft * 128 : (ft + 1) * 128],
                        xT[:, kc, tb * 512 : (tb + 1) * 512],
                        start=(kc == 0),
                        stop=(kc == 7),
                    )
                if ft < 4:
                    dst = qt[:, ft, tb * 512 : (tb + 1) * 512]
                elif ft < 8:
                    dst = kt[:, ft - 4, tb * 512 : (tb + 1) * 512]
                else:
                    dst = vt[:, ft - 8, tb * 512 : (tb + 1) * 512]
                nc.vector.tensor_scalar_add(dst, ps[:], bq[:, ft : ft + 1])

            def v_transposes(p):
                # V^T -> V (token-major) via xbar transpose, per head
                for h in (2 * p, 2 * p + 1):
                    nc.sync.dma_start_transpose(
                        v[:, h, :, 0:DK],
                        vt[(h % 2) * 64 : (h % 2) * 64 + 64, p, :],
                    )

            def fc_chain(qq, tc_i, e2):
                # tokens (qq*4 + tc_i)*128 .. +128, output cols e2*512 .. +512
                t0 = (qq * 4 + tc_i) * 128
                yp = qp.tile([128, 512], f32, tag="ps", name="yp")
                for c in range(4):
                    nc.tensor.matmul(
                        yp[:],
                        onT[:, c, t0 : t0 + 128],
                        wfc[:, c, e2 * 512 : (e2 + 1) * 512],
                        start=(c == 0),
                        stop=(c == 3),
                    )
                ys = yp_pool.tile([128, 512], bf16, tag="ys", name="ys")
                nc.vector.tensor_tensor(
                    ys[:], yp[:], bias[:, e2 * 512 : (e2 + 1) * 512], op=ADD
                )
                nc.sync.dma_start(
                    rs_in[qq][tc_i * 128 : (tc_i + 1) * 128, e2 * 512 : (e2 + 1) * 512],
                    ys[:],
                )

            def rs_quarter(qq):
                nc.gpsimd.collective_compute(
                    "ReduceScatter",
                    ADD,
                    replica_groups=PAIRS,
                    ins=[rs_in[qq].opt()],
                    outs=[rs_out[qq].opt()],
                )
                nc.gpsimd.dma_start(out[qq * Q8 : (qq + 1) * Q8, :], rs_out[qq][:])

            # ---- background work schedule (due_step, fn) ----
            bg = []
            for p in (1, 2, 3):
                items = [(0, ft, tb) for ft in (p, 4 + p) for tb in range(4)]
                items += [(1, 8 + p, tb) for tb in range(4)]
                base = 32 * (p - 1) + 2
                for i, (kind, a, b_) in enumerate(items):
                    due = base + (5 * i) // 2
                    if kind == 0:
                        bg.append((due, lambda a=a, b_=b_: qkv_chain(a, b_)))
                    else:
                        is_last = i == len(items) - 1
                        def vchain(a=a, b_=b_, p=p, is_last=is_last):
                            qkv_chain(a, b_)
                            if is_last:
                                v_transposes(p)
                        bg.append((due, vchain))
            for qq in range(3):
                items = [(tc_i, e2) for tc_i in range(4) for e2 in range(2)]
                for i, (tc_i, e2) in enumerate(items):
                    due = 128 * (qq + 1) + 4 + 7 * i
                    is_last = i == len(items) - 1
                    def fitem(qq=qq, tc_i=tc_i, e2=e2, is_last=is_last):
                        fc_chain(qq, tc_i, e2)
                        if is_last:
                            rs_quarter(qq)
                    bg.append((due, fitem))
            bg.sort(key=lambda t: t[0])
            bg_i = [0]

            def run_due_bg(s):
                while bg_i[0] < len(bg) and bg[bg_i[0]][0] <= s:
                    bg[bg_i[0]][1]()
                    bg_i[0] += 1

            # ---- prelude: pair-0 QKV so block (qq=0, h=0) can start ----
            for ft in (0, 4, 8):
                for tb in range(4):
                    qkv_chain(ft, tb)
            v_transposes(0)

            # ---- attention pipeline: 512 steps ----
            pts = {}
            av_tiles = {}

            def emit_av(s2):
                b_i, kk = divmod(s2, 16)
                qq, h = divmod(b_i, 8)
                if kk == 0:
                    av_tiles[b_i] = avp.tile([128, 512], f32, tag="av", name="av")
                av = av_tiles[b_i]
                nc.tensor.matmul(
                    av[0:65, :],
                    v[:, h, kk, 0:65],
                    pts.pop(s2)[:],
                    start=(kk == 0),
                    stop=(kk == 15),
                )
                if kk == 15:
                    evict_block(b_i, av_tiles.pop(b_i))

            def evict_block(b_i, av):
                qq, h = divmod(b_i, 8)
                j = h // 2
                qsl = slice(qq * Q4, (qq + 1) * Q4)
                srs = wp.tile([128, 1024], f32, tag="srs", name="srs")
                # sums live on PSUM partition 64; shift to partition 0 for
                # the gpsimd broadcast, reciprocal, broadcast, normalize.
                nc.vector.tensor_copy(srs[64:65, 0:512], av[64:65, :])
                nc.sync.dma_start(srs[0:1, 0:512], srs[64:65, 0:512])
                nc.vector.reciprocal_approx_fast(
                    srs[0:1, 512:1024], srs[0:1, 0:512]
                )
                R = wp.tile([128, 512], f32, tag="R", name="R")
                nc.gpsimd.partition_broadcast(R[:], srs[0:1, 512:1024])
                if h % 2 == 0:
                    nc.vector.tensor_tensor(
                        onT[0:64, j, qsl], av[0:64, :], R[0:64, :], op=MUL
                    )
                else:
                    # av rows 0:64 must land on partitions 64:128 -> DMA shift
                    tmp = wp.tile([64, 512], bf16, tag="tmp", name="tmp")
                    nc.vector.tensor_tensor(tmp[:], av[0:64, :], R[0:64, :], op=MUL)
                    nc.sync.dma_start(onT[64:128, j, qsl], tmp[:])

            for s in range(512):
                b_i, kk = divmod(s, 16)
                qq, h = divmod(b_i, 8)
                j, po = h // 2, (h % 2) * 64
                st = stp.tile([128, 512], f32, tag="st", name="st")
                nc.tensor.matmul(
                    st[:],
                    kt[po : po + 64, j, kk * 128 : (kk + 1) * 128],
                    qt[po : po + 64, j, qq * Q4 : (qq + 1) * Q4],
                    start=True,
                    stop=True,
                )
                pt = wp.tile([128, 512], bf16, tag="pt", bufs=4, name="pt")
                nc.scalar.activation(pt[:], st[:], Exp, scale=0.125)
                pts[s] = pt
                if s > 0:
                    emit_av(s - 1)
                run_due_bg(s)
            emit_av(511)

            # ---- tail: FC + RS for the last quarter ----
            run_due_bg(10**9)
            for tc_i in range(4):
                for e2 in range(2):
                    fc_chain(3, tc_i, e2)
            rs_quarter(3)

    nc.finalize()
    return nc


def _prep_inputs(X, W_qkv, b_qkv, W_fc, b_fc):
    """Host-side shard + permute + cast. Returns in_maps for 8 cores."""
    X = np.asarray(X, dtype=np.float32)
    W_qkv = np.asarray(W_qkv, dtype=np.float32)
    b_qkv = np.asarray(b_qkv, dtype=np.float32)
    W_fc = np.asarray(W_fc, dtype=np.float32)
    b_fc = np.asarray(b_fc, dtype=np.float32)

    in_maps = []
    bfc_half = (0.5 * b_fc).astype(np.float32).reshape(1, E)
    for c in range(NCORES):
        b, g = divmod(c, 2)
        heads = np.arange(g * H8, (g + 1) * H8)
        # column order: all Q feats (head-major), then K, then V
        cols = np.concatenate(
            [
                np.concatenate([h * 3 * DK + off + np.arange(DK) for h in heads])
                for off in (0, DK, 2 * DK)
            ]
        )
        wq_sh = W_qkv[:, cols].astype(ml_dtypes.bfloat16)
        bq_sh = b_qkv[cols].astype(np.float32).reshape(12, 128).T.copy()
        wfc_sh = W_fc[g * FO : (g + 1) * FO, :].astype(ml_dtypes.bfloat16)
        in_maps.append(
            {
                "x": np.ascontiguousarray(X[b].T).astype(ml_dtypes.bfloat16),
                "w_qkv": wq_sh,
                "b_qkv": np.ascontiguousarray(bq_sh),
                "w_fc": wfc_sh,
                "b_fc": bfc_half,
            }
        )
    return in_maps


def run_kernel(inputs, trace=False):
    if "nc" not in _CACHE:
        _CACHE["nc"] = build_nc()
    nc = _CACHE["nc"]
    in_maps = _prep_inputs(**inputs)
    res = bass_utils.run_bass_kernel_spmd(
        nc, in_maps, core_ids=list(range(NCORES)), trace=trace
    )
    Y = np.empty((B, L, E), dtype=np.float32)
    for c in range(NCORES):
        b, g = divmod(c, 2)
        o = np.asarray(res.results[c]["out"]).astype(np.float32)
        for qq in range(4):
            Y[b, qq * Q4 + g * Q8 : qq * Q4 + (g + 1) * Q8, :] = o[
                qq * Q8 : (qq + 1) * Q8
            ]
    return Y, res


def kernel(X, W_qkv, b_qkv, W_fc, b_fc):
    Y, _ = run_kernel(
        dict(X=X, W_qkv=W_qkv, b_qkv=b_qkv, W_fc=W_fc, b_fc=b_fc), trace=False
    )
    return Y


# revision 13
# speedup vs baseline: 1.3558x; 1.3558x over previous
"""Multi-head attention (B=4, L=2048, E=1024, H=16, DK=64) on 8 TRN2 cores.

Sharding: core c -> (batch b = c//2, head-group g = c%2 of 8 heads).

Single fused instruction stream per core, organized as one software
pipeline over 512 attention steps (4 query-quarters x 8 heads x 16
key-tiles). Per step: ST matmul (PE) -> exp (ACT) -> AV matmul (PE,
lagging one step). The scores PSUM tile is triple-buffered so ACT never
waits on a WAR hazard; QKV-projection and FC matmul chains are dripped
into the stream as background PE work between attention steps. Each
query-quarter's partial FC output is ReduceScattered (pairwise, bf16)
directly into the output tensor while later quarters compute, so only
the last quarter's FC+RS is exposed. Host casts bf16 -> f32.

Self-contained: hardcodes all shapes; requires only the concourse stack.
"""

import numpy as np
import ml_dtypes

try:
    import axon_prof

    axon_prof.install()
except Exception:
    pass

import concourse.mybir as mybir
import concourse.tile as tile
from concourse import bacc
from concourse import bass_utils

B, L, E = 4, 2048, 1024
H, DK = 16, 64
H8 = 8                      # heads per core
F = H8 * 3 * DK             # qkv features per core = 1536
FO = H8 * DK                # attn-out features per core = 512
NCORES = 8
Q4 = L // 4                 # 512 queries per quarter
Q8 = Q4 // 2                # 256 tokens scattered to each pair member

f32 = mybir.dt.float32
bf16 = mybir.dt.bfloat16
Exp = mybir.ActivationFunctionType.Exp
MUL = mybir.AluOpType.mult
ADD = mybir.AluOpType.add

_CACHE = {}


def build_nc():
    nc = bacc.Bacc("TRN2", target_bir_lowering=False, debug=False, num_devices=NCORES)

    # x arrives already transposed (host-side) so no xbar transpose is needed
    x = nc.dram_tensor("x", [E, L], bf16, kind="ExternalInput")
    w_qkv = nc.dram_tensor("w_qkv", [E, F], bf16, kind="ExternalInput")
    b_qkv = nc.dram_tensor("b_qkv", [128, 12], f32, kind="ExternalInput")
    w_fc = nc.dram_tensor("w_fc", [FO, E], bf16, kind="ExternalInput")
    b_fc = nc.dram_tensor("b_fc", [1, E], f32, kind="ExternalInput")
    # 4 quarters x 256 scattered tokens each; host casts bf16 -> f32
    out = nc.dram_tensor("out", [L // 2, E], bf16, kind="ExternalOutput")

    PAIRS = [[0, 1], [2, 3], [4, 5], [6, 7]]

    with tile.TileContext(nc) as tc:
        with (
            tc.tile_pool(name="persist", bufs=1) as pp,
            tc.tile_pool(name="work", bufs=2) as wp,
            tc.tile_pool(name="ys", bufs=3) as yp_pool,
            tc.tile_pool(name="stp", bufs=2, space="PSUM") as stp,
            tc.tile_pool(name="avp", bufs=2, space="PSUM") as avp,
            tc.tile_pool(name="qp", bufs=2, space="PSUM") as qp,
            tc.tile_pool(name="dram", bufs=1, space="DRAM") as dram,
        ):
            # ---- persistent SBUF ----
            xT = pp.tile([128, 8, L], bf16, tag="xT")          # X^T  4 MiB
            wq = pp.tile([128, 8, F], bf16, tag="wq")          # 3 MiB
            bq = pp.tile([128, 12], f32, tag="bq")
            wfc = pp.tile([128, 4, E], bf16, tag="wfc")        # 1 MiB
            bias = pp.tile([128, E], f32, tag="bias")          # 0.5 MiB
            qt = pp.tile([128, 4, L], bf16, tag="qt")          # Q^T 2 MiB
            kt = pp.tile([128, 4, L], bf16, tag="kt")          # K^T 2 MiB
            vt = pp.tile([128, 4, L], bf16, tag="vt")          # V^T 2 MiB
            # V natural layout, 80-elem stride; col 64 holds the ones column
            # so AV matmuls with lhsT [V|1] (M=65) produce rowsums for free
            v = pp.tile([128, H8, 16, 80], bf16, tag="v")      # 2.5 MiB
            onT = pp.tile([128, 4, L], bf16, tag="onT")        # attn out^T 2 MiB

            rs_in = [
                dram.tile([Q4, E], bf16, name=f"rs_in{i}", tag=f"rs_in{i}")
                for i in range(4)
            ]
            rs_out = [
                dram.tile([Q8, E], bf16, name=f"rs_out{i}", tag=f"rs_out{i}")
                for i in range(4)
            ]

            # ---- input DMAs over both HWDGE queues ----
            queues = [nc.sync, nc.scalar]
            for e in range(8):
                queues[e % 2].dma_start(xT[:, e, :], x[e * 128 : (e + 1) * 128, :])
            for e in range(8):
                queues[(e + 1) % 2].dma_start(
                    wq[:, e, :], w_qkv[e * 128 : (e + 1) * 128, :]
                )
            nc.scalar.dma_start(bq[:], b_qkv[:])
            nc.scalar.dma_start(wfc[:], w_fc.rearrange("(c p) e -> p c e", p=128))
            bfc_row = pp.tile([1, E], f32, tag="bfc_row")
            nc.scalar.dma_start(bfc_row[:], b_fc[:])
            nc.gpsimd.partition_broadcast(bias[:], bfc_row[:])
            nc.vector.memset(v[:, :, :, 64:65], 1.0)

            # ---- instruction emitters ----
            # every matmul in this kernel uses PE tile shape (64, 128): the
            # PE pays a large reconfig penalty on tile-shape switches, and
            # K=64 bf16 streams 2 moving rows/cycle so the split is free.
            def qkv_chain(ft, tb):
                ps = qp.tile([128, 512], f32, tag="ps", name="ps")
                for kc in range(8):
                    nc.tensor.matmul(
                        ps[:],
                        wq[:, kc, ft * 128 : (ft + 1) * 128],
                        xT[:, kc, tb * 512 : (tb + 1) * 512],
                        start=(kc == 0),
                        stop=(kc == 7),
                    )
                if ft < 4:
                    dst = qt[:, ft, tb * 512 : (tb + 1) * 512]
                elif ft < 8:
                    dst = kt[:, ft - 4, tb * 512 : (tb + 1) * 512]
                else:
                    dst = vt[:, ft - 8, tb * 512 : (tb + 1) * 512]
                nc.vector.tensor_scalar_add(dst, ps[:], bq[:, ft : ft + 1])

            def v_transposes(p):
                # V^T -> V (token-major) via xbar transpose, per head
                for h in (2 * p, 2 * p + 1):
                    nc.sync.dma_start_transpose(
                        v[:, h, :, 0:DK],
                        vt[(h % 2) * 64 : (h % 2) * 64 + 64, p, :],
                    )

            def fc_chain(qq, tc_i, e2):
                # tokens (qq*4 + tc_i)*128 .. +128, output cols e2*512 .. +512
                t0 = (qq * 4 + tc_i) * 128
                yp = qp.tile([128, 512], f32, tag="ps", name="yp")
                for c in range(4):
                    nc.tensor.matmul(
                        yp[:],
                        onT[:, c, t0 : t0 + 128],
                        wfc[:, c, e2 * 512 : (e2 + 1) * 512],
                        start=(c == 0),
                        stop=(c == 3),
                    )
                ys = yp_pool.tile([128, 512], bf16, tag="ys", name="ys")
                nc.vector.tensor_tensor(
                    ys[:], yp[:], bias[:, e2 * 512 : (e2 + 1) * 512], op=ADD
                )
                nc.sync.dma_start(
                    rs_in[qq][tc_i * 128 : (tc_i + 1) * 128, e2 * 512 : (e2 + 1) * 512],
                    ys[:],
                )

            def rs_quarter(qq):
                nc.gpsimd.collective_compute(
                    "ReduceScatter",
                    ADD,
                    replica_groups=PAIRS,
                    ins=[rs_in[qq].opt()],
                    outs=[rs_out[qq].opt()],
                )
                nc.gpsimd.dma_start(out[qq * Q8 : (qq + 1) * Q8, :], rs_out[qq][:])

            # ---- background work schedule (due_pair_step, fn) ----
            bg = []
            for p in (1, 2, 3):
                items = [(0, ft, tb) for ft in (p, 4 + p) for tb in range(4)]
                items += [(1, 8 + p, tb) for tb in range(4)]
                base = 16 * (p - 1) + 1
                for i, (kind, a, b_) in enumerate(items):
                    due = base + (5 * i) // 4
                    if kind == 0:
                        bg.append((due, lambda a=a, b_=b_: qkv_chain(a, b_)))
                    else:
                        is_last = i == len(items) - 1
                        def vchain(a=a, b_=b_, p=p, is_last=is_last):
                            qkv_chain(a, b_)
                            if is_last:
                                v_transposes(p)
                        bg.append((due, vchain))
            for qq in range(3):
                items = [(tc_i, e2) for tc_i in range(4) for e2 in range(2)]
                for i, (tc_i, e2) in enumerate(items):
                    due = 64 * (qq + 1) + 2 + 6 * i
                    is_last = i == len(items) - 1
                    def fitem(qq=qq, tc_i=tc_i, e2=e2, is_last=is_last):
                        fc_chain(qq, tc_i, e2)
                        if is_last:
                            rs_quarter(qq)
                    bg.append((due, fitem))
            bg.sort(key=lambda t: t[0])
            bg_i = [0]

            def run_due_bg(s):
                while bg_i[0] < len(bg) and bg[bg_i[0]][0] <= s:
                    bg[bg_i[0]][1]()
                    bg_i[0] += 1

            # ---- prelude: pair-0 QKV so block (qq=0, h=0) can start ----
            for ft in (0, 4, 8):
                for tb in range(4):
                    qkv_chain(ft, tb)
            v_transposes(0)

            # ---- attention pipeline: 256 pair-steps (2 key-tiles each) ----
            pts = {}
            av_tiles = {}

            def emit_av(pp_):
                b_i, kp = divmod(pp_, 8)
                qq, h = divmod(b_i, 8)
                if kp == 0:
                    av_tiles[b_i] = avp.tile([128, 512], f32, tag="av", name="av")
                av = av_tiles[b_i]
                pt = pts.pop(pp_)
                for kk in (2 * kp, 2 * kp + 1):
                    nc.tensor.matmul(
                        av[0:65, :],
                        v[:, h, kk, 0:65],
                        pt[:, (kk % 2) * 512 : (kk % 2) * 512 + 512],
                        start=(kk == 0),
                        stop=(kk == 15),
                    )
                if kp == 7:
                    evict_block(b_i, av_tiles.pop(b_i))

            def evict_block(b_i, av):
                qq, h = divmod(b_i, 8)
                j = h // 2
                qsl = slice(qq * Q4, (qq + 1) * Q4)
                srs = wp.tile([128, 1024], f32, tag="srs", name="srs")
                # sums live on PSUM partition 64; shift to partition 0 for
                # the gpsimd broadcast, reciprocal, broadcast, normalize.
                nc.vector.tensor_copy(srs[64:65, 0:512], av[64:65, :])
                nc.sync.dma_start(srs[0:1, 0:512], srs[64:65, 0:512])
                nc.vector.reciprocal_approx_fast(
                    srs[0:1, 512:1024], srs[0:1, 0:512]
                )
                R = wp.tile([128, 512], f32, tag="R", name="R")
                nc.gpsimd.partition_broadcast(R[:], srs[0:1, 512:1024])
                if h % 2 == 0:
                    nc.vector.tensor_tensor(
                        onT[0:64, j, qsl], av[0:64, :], R[0:64, :], op=MUL
                    )
                else:
                    # av rows 0:64 must land on partitions 64:128 -> DMA shift
                    tmp = wp.tile([64, 512], bf16, tag="tmp", name="tmp")
                    nc.vector.tensor_tensor(tmp[:], av[0:64, :], R[0:64, :], op=MUL)
                    nc.sync.dma_start(onT[64:128, j, qsl], tmp[:])

            for pp_ in range(256):
                b_i, kp = divmod(pp_, 8)
                qq, h = divmod(b_i, 8)
                j, po = h // 2, (h % 2) * 64
                st = stp.tile([128, 1024], f32, tag="st", name="st")
                for kk in (2 * kp, 2 * kp + 1):
                    nc.tensor.matmul(
                        st[:, (kk % 2) * 512 : (kk % 2) * 512 + 512],
                        kt[po : po + 64, j, kk * 128 : (kk + 1) * 128],
                        qt[po : po + 64, j, qq * Q4 : (qq + 1) * Q4],
                        start=True,
                        stop=True,
                    )
                pt = wp.tile([128, 1024], bf16, tag="pt", bufs=3, name="pt")
                nc.scalar.activation(pt[:], st[:], Exp, scale=0.125)
                pts[pp_] = pt
                if pp_ > 0:
                    emit_av(pp_ - 1)
                run_due_bg(pp_)
            emit_av(255)

            # ---- tail: FC + RS for the last quarter ----
            run_due_bg(10**9)
            for tc_i in range(4):
                for e2 in range(2):
                    fc_chain(3, tc_i, e2)
            rs_quarter(3)

    nc.finalize()
    return nc


def _prep_inputs(X, W_qkv, b_qkv, W_fc, b_fc):
    """Host-side shard + permute + cast. Returns in_maps for 8 cores."""
    X = np.asarray(X, dtype=np.float32)
    W_qkv = np.asarray(W_qkv, dtype=np.float32)
    b_qkv = np.asarray(b_qkv, dtype=np.float32)
    W_fc = np.asarray(W_fc, dtype=np.float32)
    b_fc = np.asarray(b_fc, dtype=np.float32)

    in_maps = []
    bfc_half = (0.5 * b_fc).astype(np.float32).reshape(1, E)
    for c in range(NCORES):
        b, g = divmod(c, 2)
        heads = np.arange(g * H8, (g + 1) * H8)
        # column order: all Q feats (head-major), then K, then V
        cols = np.concatenate(
            [
                np.concatenate([h * 3 * DK + off + np.arange(DK) for h in heads])
                for off in (0, DK, 2 * DK)
            ]
        )
        wq_sh = W_qkv[:, cols].astype(ml_dtypes.bfloat16)
        bq_sh = b_qkv[cols].astype(np.float32).reshape(12, 128).T.copy()
        wfc_sh = W_fc[g * FO : (g + 1) * FO, :].astype(ml_dtypes.bfloat16)
        in_maps.append(
            {
                "x": np.ascontiguousarray(X[b].T).astype(ml_dtypes.bfloat16),
                "w_qkv": wq_sh,
                "b_qkv": np.ascontiguousarray(bq_sh),
                "w_fc": wfc_sh,
                "b_fc": bfc_half,
            }
        )
    return in_maps


def run_kernel(inputs, trace=False):
    if "nc" not in _CACHE:
        _CACHE["nc"] = build_nc()
    nc = _CACHE["nc"]
    in_maps = _prep_inputs(**inputs)
    res = bass_utils.run_bass_kernel_spmd(
        nc, in_maps, core_ids=list(range(NCORES)), trace=trace
    )
    Y = np.empty((B, L, E), dtype=np.float32)
    for c in range(NCORES):
        b, g = divmod(c, 2)
        o = np.asarray(res.results[c]["out"]).astype(np.float32)
        for qq in range(4):
            Y[b, qq * Q4 + g * Q8 : qq * Q4 + (g + 1) * Q8, :] = o[
                qq * Q8 : (qq + 1) * Q8
            ]
    return Y, res


def kernel(X, W_qkv, b_qkv, W_fc, b_fc):
    Y, _ = run_kernel(
        dict(X=X, W_qkv=W_qkv, b_qkv=b_qkv, W_fc=W_fc, b_fc=b_fc), trace=False
    )
    return Y


# revision 20
# speedup vs baseline: 1.3777x; 1.0162x over previous
"""Multi-head attention (B=4, L=2048, E=1024, H=16, DK=64) on 8 TRN2 cores.

Sharding: core c -> (batch b = c//2, head-group g = c%2 of 8 heads).

Single fused instruction stream per core, organized as one software
pipeline over 512 attention steps (4 query-quarters x 8 heads x 16
key-tiles). Per step: ST matmul (PE) -> exp (ACT) -> AV matmul (PE,
lagging one step). The scores PSUM tile is triple-buffered so ACT never
waits on a WAR hazard; QKV-projection and FC matmul chains are dripped
into the stream as background PE work between attention steps. Each
query-quarter's partial FC output is ReduceScattered (pairwise, bf16)
directly into the output tensor while later quarters compute, so only
the last quarter's FC+RS is exposed. Host casts bf16 -> f32.

Self-contained: hardcodes all shapes; requires only the concourse stack.
"""

import numpy as np
import ml_dtypes

try:
    import axon_prof

    axon_prof.install()
except Exception:
    pass

import concourse.mybir as mybir
import concourse.tile as tile
from concourse import bacc
from concourse import bass_utils

B, L, E = 4, 2048, 1024
H, DK = 16, 64
H8 = 8                      # heads per core
F = H8 * 3 * DK             # qkv features per core = 1536
FO = H8 * DK                # attn-out features per core = 512
NCORES = 8
Q4 = L // 4                 # 512 queries per quarter
Q8 = Q4 // 2                # 256 tokens scattered to each pair member

f32 = mybir.dt.float32
bf16 = mybir.dt.bfloat16
Exp = mybir.ActivationFunctionType.Exp
MUL = mybir.AluOpType.mult
ADD = mybir.AluOpType.add

_CACHE = {}


def build_nc():
    nc = bacc.Bacc("TRN2", target_bir_lowering=False, debug=False, num_devices=NCORES)

    # x arrives already transposed (host-side) so no xbar transpose is needed
    x = nc.dram_tensor("x", [E, L], bf16, kind="ExternalInput")
    w_qkv = nc.dram_tensor("w_qkv", [E, F], bf16, kind="ExternalInput")
    b_qkv = nc.dram_tensor("b_qkv", [128, 12], f32, kind="ExternalInput")
    w_fc = nc.dram_tensor("w_fc", [FO, E], bf16, kind="ExternalInput")
    b_fc = nc.dram_tensor("b_fc", [1, E], f32, kind="ExternalInput")
    # 4 quarters x 256 scattered tokens each; host casts bf16 -> f32
    out = nc.dram_tensor("out", [L // 2, E], bf16, kind="ExternalOutput")

    PAIRS = [[0, 1], [2, 3], [4, 5], [6, 7]]

    with tile.TileContext(nc) as tc:
        with (
            tc.tile_pool(name="persist", bufs=1) as pp,
            tc.tile_pool(name="work", bufs=2) as wp,
            tc.tile_pool(name="ys", bufs=3) as yp_pool,
            tc.tile_pool(name="stp", bufs=2, space="PSUM") as stp,
            tc.tile_pool(name="avp", bufs=1, space="PSUM") as avp,
            tc.tile_pool(name="qp", bufs=2, space="PSUM") as qp,
            tc.tile_pool(name="dram", bufs=1, space="DRAM") as dram,
        ):
            # ---- persistent SBUF ----
            xT = pp.tile([128, 8, L], bf16, tag="xT")          # X^T  4 MiB
            wq = pp.tile([128, 8, F], bf16, tag="wq")          # 3 MiB
            bq = pp.tile([128, 12], f32, tag="bq")
            wfc = pp.tile([128, 4, E], bf16, tag="wfc")        # 1 MiB
            bias = pp.tile([128, E], f32, tag="bias")          # 0.5 MiB
            qt = pp.tile([128, 4, L], bf16, tag="qt")          # Q^T 2 MiB
            kt = pp.tile([128, 4, L], bf16, tag="kt")          # K^T 2 MiB
            vt = pp.tile([128, 4, L], bf16, tag="vt")          # V^T 2 MiB
            # V natural layout, 80-elem stride; col 64 holds the ones column
            # so AV matmuls with lhsT [V|1] (M=65) produce rowsums for free
            v = pp.tile([128, H8, 16, 80], bf16, tag="v")      # 2.5 MiB
            onT = pp.tile([128, 4, L], bf16, tag="onT")        # attn out^T 2 MiB

            rs_in = [
                dram.tile([Q4, E], bf16, name=f"rs_in{i}", tag=f"rs_in{i}")
                for i in range(4)
            ]
            rs_out = [
                dram.tile([Q8, E], bf16, name=f"rs_out{i}", tag=f"rs_out{i}")
                for i in range(4)
            ]

            # ---- input DMAs: X on sync; wq by ft-column groups on scalar so
            # the first attention blocks' weights arrive in ~2us ----
            for e in range(8):
                nc.sync.dma_start(xT[:, e, :], x[e * 128 : (e + 1) * 128, :])
            wq_src = w_qkv.rearrange("(a p) f -> p a f", p=128)
            for ft in (0, 4, 8, 1, 5, 9, 2, 6, 10, 3, 7, 11):
                nc.scalar.dma_start(
                    wq[:, :, ft * 128 : (ft + 1) * 128],
                    wq_src[:, :, ft * 128 : (ft + 1) * 128],
                )
                if ft == 8:
                    nc.scalar.dma_start(bq[:], b_qkv[:])
            nc.scalar.dma_start(wfc[:], w_fc.rearrange("(c p) e -> p c e", p=128))
            bfc_row = pp.tile([1, E], f32, tag="bfc_row")
            nc.scalar.dma_start(bfc_row[:], b_fc[:])
            nc.gpsimd.partition_broadcast(bias[:], bfc_row[:])
            nc.vector.memset(v[:, :, :, 64:65], 1.0)

            # ---- instruction emitters ----
            # every matmul in this kernel uses PE tile shape (64, 128): the
            # PE pays a large reconfig penalty on tile-shape switches, and
            # K=64 bf16 streams 2 moving rows/cycle so the split is free.
            def qkv_chain(ft, tb):
                ps = qp.tile([128, 512], f32, tag="ps", name="ps")
                for kc in range(8):
                    nc.tensor.matmul(
                        ps[:],
                        wq[:, kc, ft * 128 : (ft + 1) * 128],
                        xT[:, kc, tb * 512 : (tb + 1) * 512],
                        start=(kc == 0),
                        stop=(kc == 7),
                    )
                if ft < 4:
                    dst = qt[:, ft, tb * 512 : (tb + 1) * 512]
                elif ft < 8:
                    dst = kt[:, ft - 4, tb * 512 : (tb + 1) * 512]
                else:
                    dst = vt[:, ft - 8, tb * 512 : (tb + 1) * 512]
                nc.vector.tensor_scalar_add(dst, ps[:], bq[:, ft : ft + 1])

            def v_transpose(p, tb):
                # V^T -> V (token-major) via xbar transpose, per head, per
                # 512-token slice (4 key-chunks)
                for h in (2 * p, 2 * p + 1):
                    nc.sync.dma_start_transpose(
                        v[:, h, tb * 4 : (tb + 1) * 4, 0:DK],
                        vt[(h % 2) * 64 : (h % 2) * 64 + 64, p, tb * 512 : (tb + 1) * 512],
                    )

            def fc_chain(qq, tc_i, e2):
                # tokens (qq*4 + tc_i)*128 .. +128, output cols e2*512 .. +512
                t0 = (qq * 4 + tc_i) * 128
                yp = qp.tile([128, 512], f32, tag="ps", name="yp")
                for c in range(4):
                    nc.tensor.matmul(
                        yp[:],
                        onT[:, c, t0 : t0 + 128],
                        wfc[:, c, e2 * 512 : (e2 + 1) * 512],
                        start=(c == 0),
                        stop=(c == 3),
                    )
                ys = yp_pool.tile([128, 512], bf16, tag="ys", name="ys")
                nc.vector.tensor_tensor(
                    ys[:], yp[:], bias[:, e2 * 512 : (e2 + 1) * 512], op=ADD
                )
                nc.sync.dma_start(
                    rs_in[qq][tc_i * 128 : (tc_i + 1) * 128, e2 * 512 : (e2 + 1) * 512],
                    ys[:],
                )

            def rs_quarter(qq):
                nc.gpsimd.collective_compute(
                    "ReduceScatter",
                    ADD,
                    replica_groups=PAIRS,
                    ins=[rs_in[qq].opt()],
                    outs=[rs_out[qq].opt()],
                )
                nc.gpsimd.dma_start(out[qq * Q8 : (qq + 1) * Q8, :], rs_out[qq][:])

            # ---- background work schedule (due_pair_step, fn) ----
            # dues are set just before each tile's first consuming pair-step;
            # run_due_bg() fires at the TOP of a pair-step, before its STs.
            bg = []

            def vitem(p, tb):
                def f():
                    qkv_chain(8 + p, tb)
                    v_transpose(p, tb)
                return f

            # head-pair 0 tail (tb 1..3): consumed at pairs 2tb, 2tb+1
            for tb in (1, 2, 3):
                bg.append((2 * tb - 1, lambda tb=tb: qkv_chain(4, tb)))
                bg.append((2 * tb, vitem(0, tb)))
            # head-pairs 1..3: block (qq=0, h=2p) starts at pair 16p
            for p in (1, 2, 3):
                bg.append((16 * p - 4, lambda p=p: qkv_chain(p, 0)))
                for tb in range(4):
                    bg.append((16 * p + 2 * tb - 2, lambda p=p, tb=tb: qkv_chain(4 + p, tb)))
                    bg.append((16 * p + 2 * tb - 1, vitem(p, tb)))
            # Q projections for quarters 1..3 (consumed at pair 64*tb + 16*p)
            for tb in (1, 2, 3):
                for p in range(4):
                    bg.append((64 * tb + 16 * p - 4, lambda p=p, tb=tb: qkv_chain(p, tb)))
            # FC + RS for quarters 0..2 (quarter qq done after pair 64*qq+63)
            for qq in range(3):
                items = [(tc_i, e2) for tc_i in range(4) for e2 in range(2)]
                for i, (tc_i, e2) in enumerate(items):
                    due = 64 * (qq + 1) + 2 + 6 * i
                    is_last = i == len(items) - 1
                    def fitem(qq=qq, tc_i=tc_i, e2=e2, is_last=is_last):
                        fc_chain(qq, tc_i, e2)
                        if is_last:
                            rs_quarter(qq)
                    bg.append((due, fitem))
            bg.sort(key=lambda t: t[0])
            bg_i = [0]

            def run_due_bg(s):
                while bg_i[0] < len(bg) and bg[bg_i[0]][0] <= s:
                    bg[bg_i[0]][1]()
                    bg_i[0] += 1

            # ---- prelude: just enough QKV for pairs 0..1 of block (0,0) ----
            qkv_chain(0, 0)
            qkv_chain(4, 0)
            qkv_chain(8, 0)
            v_transpose(0, 0)

            # ---- attention pipeline: 256 pair-steps (2 key-tiles each) ----
            pts = {}
            av_tiles = {}

            def emit_av(pp_):
                b_i, kp = divmod(pp_, 8)
                qq, h = divmod(b_i, 8)
                if kp == 0:
                    av_tiles[b_i] = (
                        avp.tile([128, 512], f32, tag="avA", name="avA"),
                        avp.tile([128, 512], f32, tag="avB", name="avB"),
                    )
                avA, avB = av_tiles[b_i]
                pt = pts.pop(pp_)
                # two accumulation groups, each with a fixed PE tile position:
                # avA sums keys [0:64) of each tile, avB keys [64:128)
                for po2, av in ((0, avA), (64, avB)):
                    for kk in (2 * kp, 2 * kp + 1):
                        nc.tensor.matmul(
                            av[0:65, :],
                            v[po2 : po2 + 64, h, kk, 0:65],
                            pt[po2 : po2 + 64, (kk % 2) * 512 : (kk % 2) * 512 + 512],
                            start=(kk == 2 * kp and kp == 0),
                            stop=(kk == 2 * kp + 1 and kp == 7),
                        )
                if kp == 7:
                    evict_block(b_i, *av_tiles.pop(b_i))

            def evict_block(b_i, avA, avB):
                qq, h = divmod(b_i, 8)
                j = h // 2
                qsl = slice(qq * Q4, (qq + 1) * Q4)
                # DVE reads at most one PSUM operand per op: copy then add
                comb = wp.tile([128, 512], f32, tag="comb", name="comb")
                nc.vector.tensor_copy(comb[0:65, :], avA[0:65, :])
                nc.vector.tensor_tensor(
                    comb[0:65, :], comb[0:65, :], avB[0:65, :], op=ADD
                )
                # sums live on partition 64; shift to partition 0 for the
                # gpsimd broadcast, then reciprocal, broadcast, normalize.
                srs = wp.tile([128, 1024], f32, tag="srs", name="srs")
                nc.sync.dma_start(srs[0:1, 0:512], comb[64:65, :])
                nc.vector.reciprocal_approx_fast(
                    srs[0:1, 512:1024], srs[0:1, 0:512]
                )
                R = wp.tile([128, 512], f32, tag="R", name="R")
                nc.gpsimd.partition_broadcast(R[:], srs[0:1, 512:1024])
                if h % 2 == 0:
                    nc.vector.tensor_tensor(
                        onT[0:64, j, qsl], comb[0:64, :], R[0:64, :], op=MUL
                    )
                else:
                    # rows 0:64 must land on partitions 64:128 -> DMA shift
                    tmp = wp.tile([64, 512], bf16, tag="tmp", name="tmp")
                    nc.vector.tensor_tensor(tmp[:], comb[0:64, :], R[0:64, :], op=MUL)
                    nc.sync.dma_start(onT[64:128, j, qsl], tmp[:])

            for pp_ in range(256):
                b_i, kp = divmod(pp_, 8)
                qq, h = divmod(b_i, 8)
                j, po = h // 2, (h % 2) * 64
                st = stp.tile([128, 1024], f32, tag="st", name="st")
                for kk in (2 * kp, 2 * kp + 1):
                    nc.tensor.matmul(
                        st[:, (kk % 2) * 512 : (kk % 2) * 512 + 512],
                        kt[po : po + 64, j, kk * 128 : (kk + 1) * 128],
                        qt[po : po + 64, j, qq * Q4 : (qq + 1) * Q4],
                        start=True,
                        stop=True,
                    )
                pt = wp.tile([128, 1024], bf16, tag="pt", bufs=3, name="pt")
                nc.scalar.activation(pt[:], st[:], Exp, scale=0.125)
                pts[pp_] = pt
                if pp_ > 0:
                    emit_av(pp_ - 1)
                run_due_bg(pp_)
            emit_av(255)

            # ---- tail: FC + RS for the last quarter ----
            run_due_bg(10**9)
            for tc_i in range(4):
                for e2 in range(2):
                    fc_chain(3, tc_i, e2)
            rs_quarter(3)

    nc.finalize()
    return nc


def _prep_inputs(X, W_qkv, b_qkv, W_fc, b_fc):
    """Host-side shard + permute + cast. Returns in_maps for 8 cores."""
    X = np.asarray(X, dtype=np.float32)
    W_qkv = np.asarray(W_qkv, dtype=np.float32)
    b_qkv = np.asarray(b_qkv, dtype=np.float32)
    W_fc = np.asarray(W_fc, dtype=np.float32)
    b_fc = np.asarray(b_fc, dtype=np.float32)

    in_maps = []
    bfc_half = (0.5 * b_fc).astype(np.float32).reshape(1, E)
    for c in range(NCORES):
        b, g = divmod(c, 2)
        heads = np.arange(g * H8, (g + 1) * H8)
        # column order: all Q feats (head-major), then K, then V
        cols = np.concatenate(
            [
                np.concatenate([h * 3 * DK + off + np.arange(DK) for h in heads])
                for off in (0, DK, 2 * DK)
            ]
        )
        wq_sh = W_qkv[:, cols].astype(ml_dtypes.bfloat16)
        bq_sh = b_qkv[cols].astype(np.float32).reshape(12, 128).T.copy()
        wfc_sh = W_fc[g * FO : (g + 1) * FO, :].astype(ml_dtypes.bfloat16)
        in_maps.append(
            {
                "x": np.ascontiguousarray(X[b].T).astype(ml_dtypes.bfloat16),
                "w_qkv": wq_sh,
                "b_qkv": np.ascontiguousarray(bq_sh),
                "w_fc": wfc_sh,
                "b_fc": bfc_half,
            }
        )
    return in_maps


def run_kernel(inputs, trace=False):
    if "nc" not in _CACHE:
        _CACHE["nc"] = build_nc()
    nc = _CACHE["nc"]
    in_maps = _prep_inputs(**inputs)
    res = bass_utils.run_bass_kernel_spmd(
        nc, in_maps, core_ids=list(range(NCORES)), trace=trace
    )
    Y = np.empty((B, L, E), dtype=np.float32)
    for c in range(NCORES):
        b, g = divmod(c, 2)
        o = np.asarray(res.results[c]["out"]).astype(np.float32)
        for qq in range(4):
            Y[b, qq * Q4 + g * Q8 : qq * Q4 + (g + 1) * Q8, :] = o[
                qq * Q8 : (qq + 1) * Q8
            ]
    return Y, res


def kernel(X, W_qkv, b_qkv, W_fc, b_fc):
    Y, _ = run_kernel(
        dict(X=X, W_qkv=W_qkv, b_qkv=b_qkv, W_fc=W_fc, b_fc=b_fc), trace=False
    )
    return Y
